# revision 19
# baseline (speedup 1.0000x reference)
"""AttentionBlock (GroupNorm -> qkv conv1x1 -> 4-head attention -> proj + residual)
on 8 Trainium2 NeuronCores.

Sharding: B*NH = 2*4 = 8 (batch, head) pairs -> one per core.
Each core:
  - GroupNorm(32, 512) over its batch's x (recomputed per core; vector work)
  - qkv for its head:  q,k,v = W'[3*128, 512] @ xn   (norm affine + qk scale
    folded into W'/bias on host)
  - scoresT[s,t] = sum_c k[c,s] q[c,t]  (s on partitions -> exp output needs
    no transposes).  No max-subtraction: scores are O(1) for this problem.
  - eT = exp(scoresT) in fp16;  Z[t] via fp16 pairwise add-tree + ones-matmul
  - h[c,t] = (sum_s v[c,s] eT[s,t]) * (1/Z[t])
  - partial[o,t] = w_proj[o, head_slice] @ h
Host: out[b] = sum_heads partial + b_proj + x  (gather/unshard).
"""

import math
from contextlib import ExitStack

import numpy as np

import concourse.bacc as bacc
import concourse.bass as bass
import concourse.mybir as mybir
import concourse.tile as tile
from concourse.bass_utils import run_bass_kernel_spmd
from concourse.masks import make_identity

C = 512
NH = 4
G = 32
EPS = 1e-5
N = 4096          # H*W
CH = 128          # channels per head
B = 2
NCORES = 8
TCHUNK = 1024     # t-columns processed per chunk
NCHUNK = N // TCHUNK
NST = N // 128    # number of 128-wide s tiles

F16 = mybir.dt.float16
F32 = mybir.dt.float32

TRACE = False
TRACE_CORES = [0]
LAST_RESULT = None


def build_program():
    nc = bacc.Bacc()

    x16 = nc.declare_dram_parameter("x16", [C, N], F16, isOutput=False)
    wqkvT = nc.declare_dram_parameter("wqkvT", [4, 128, 3 * CH], F16, isOutput=False)
    bqkv = nc.declare_dram_parameter("bqkv", [128, 3], F32, isOutput=False)
    wprojT = nc.declare_dram_parameter("wprojT", [CH, C], F16, isOutput=False)
    # group membership matrices: mgrp[p, g] = (p // 16 == g)
    mgrp = nc.declare_dram_parameter("mgrp", [128, 8], F16, isOutput=False)
    mgrpT = nc.declare_dram_parameter("mgrpT", [8, 128], F16, isOutput=False)
    partial = nc.declare_dram_parameter("partial", [C, N], F32, isOutput=True)

    with tile.TileContext(nc) as tc, ExitStack() as ctx:
        consts = ctx.enter_context(tc.tile_pool(name="consts", bufs=1))
        gn = ctx.enter_context(tc.tile_pool(name="gn", bufs=1))
        xpool = ctx.enter_context(tc.tile_pool(name="xpool", bufs=4))
        spool = ctx.enter_context(tc.tile_pool(name="spool", bufs=2))
        qkvp = ctx.enter_context(tc.tile_pool(name="qkvp", bufs=1))
        epool = ctx.enter_context(tc.tile_pool(name="epool", bufs=30))
        trpool = ctx.enter_context(tc.tile_pool(name="trpool", bufs=5))
        gspool = ctx.enter_context(tc.tile_pool(name="gspool", bufs=5))
        espool = ctx.enter_context(tc.tile_pool(name="espool", bufs=2))
        zpool = ctx.enter_context(tc.tile_pool(name="zpool", bufs=2))
        hpool = ctx.enter_context(tc.tile_pool(name="hpool", bufs=3))
        opool = ctx.enter_context(tc.tile_pool(name="opool", bufs=3))
        ps_sc = ctx.enter_context(tc.tile_pool(name="ps_sc", bufs=2, space="PSUM"))
        ps_acc = ctx.enter_context(tc.tile_pool(name="ps_acc", bufs=2, space="PSUM"))
        ps_mm2 = ctx.enter_context(tc.tile_pool(name="ps_mm2", bufs=2, space="PSUM"))

        # ---- constants ----
        ident128_16 = consts.tile([128, 128], F16, tag="id16")
        make_identity(nc, ident128_16)
        mgrp_sb = consts.tile([128, 8], F16, tag="mgrp")
        nc.sync.dma_start(out=mgrp_sb, in_=mgrp[:, :])
        mgrpT_sb = consts.tile([8, 128], F16, tag="mgrpT")
        nc.sync.dma_start(out=mgrpT_sb, in_=mgrpT[:, :])
        ones_col = consts.tile([128, 1], F16, tag="ones")
        nc.vector.memset(ones_col, 1.0)
        eps_sb = consts.tile([128, 1], F32, tag="eps")
        nc.vector.memset(eps_sb, EPS)

        w_tiles = []
        for kt in range(4):
            wt = consts.tile([128, 3 * CH], F16, tag=f"wq{kt}")
            nc.sync.dma_start(out=wt, in_=wqkvT[kt])
            w_tiles.append(wt)
        bq_sb = consts.tile([128, 3], F32, tag="bq")
        nc.sync.dma_start(out=bq_sb, in_=bqkv[:, :])
        wp_sb = consts.tile([CH, C], F16, tag="wp")
        nc.sync.dma_start(out=wp_sb, in_=wprojT[:, :])

        # ---- load x tiles + per-channel stats ----
        stats_all = gn.tile([128, 8], F32, tag="stats_all")
        xt = []
        for i in range(4):
            xti = xpool.tile([128, N], F16, tag="xt")
            nc.sync.dma_start(out=xti, in_=x16[128 * i : 128 * (i + 1), :])
            xt.append(xti)
            st = spool.tile([128, 8, 6], F32, tag="bst")
            xv = xti.rearrange("p (s f) -> p s f", f=512)
            for s in range(8):
                nc.vector.bn_stats(out=st[:, s, :], in_=xv[:, s, :])
            mv = spool.tile([128, 2], F32, tag="mv")
            nc.vector.bn_aggr(out=mv, in_=st)
            # stats_all[:, i] = channel mean;  stats_all[:, 4+i] = E[x^2]
            nc.vector.tensor_copy(out=stats_all[:, i : i + 1], in_=mv[:, 0:1])
            nc.vector.tensor_mul(
                out=stats_all[:, 4 + i : 5 + i], in0=mv[:, 0:1], in1=mv[:, 0:1]
            )
            nc.vector.tensor_add(
                out=stats_all[:, 4 + i : 5 + i],
                in0=stats_all[:, 4 + i : 5 + i],
                in1=mv[:, 1:2],
            )

        # ---- cross-partition group aggregation via PE ----
        # gs[g, col] = sum over partitions p in group g of stats_all[p, col]
        stats16 = gn.tile([128, 8], F16, tag="stats16")
        nc.vector.tensor_copy(out=stats16, in_=stats_all)
        ps_t = ps_mm2.tile([8, 8], F32, tag="mm2")
        nc.tensor.matmul(ps_t, lhsT=mgrp_sb, rhs=stats16, start=True, stop=True)
        gs = gn.tile([8, 8], F32, tag="gs8")
        nc.scalar.mul(out=gs, in_=ps_t, mul=1.0 / 16.0)
        # gvals cols 0..3 = group mean per x-tile, cols 4..7 = group rstd
        gvals = gn.tile([8, 8], F32, tag="gvals")
        nc.vector.tensor_copy(out=gvals[:, 0:4], in_=gs[:, 0:4])
        varg = gn.tile([8, 4], F32, tag="varg")
        nc.vector.tensor_mul(out=varg, in0=gs[:, 0:4], in1=gs[:, 0:4])  # mu^2
        nc.vector.tensor_sub(out=varg, in0=gs[:, 4:8], in1=varg)  # var
        nc.scalar.activation(
            out=varg,
            in_=varg,
            func=mybir.ActivationFunctionType.Sqrt,
            bias=eps_sb[0:8, :],
        )
        nc.vector.reciprocal(out=gvals[:, 4:8], in_=varg)  # rstd
        # broadcast group values back to all 128 channel partitions
        gvals16 = gn.tile([8, 8], F16, tag="gvals16")
        nc.vector.tensor_copy(out=gvals16, in_=gvals)
        ps_t2 = ps_mm2.tile([128, 8], F32, tag="mm2")
        nc.tensor.matmul(ps_t2, lhsT=mgrpT_sb, rhs=gvals16, start=True, stop=True)
        sc_all = gn.tile([128, 8], F32, tag="scall")
        nc.vector.tensor_copy(out=sc_all, in_=ps_t2)

        # ---- apply normalization in place: xn = (x - mu) * rstd ----
        for i in range(4):
            nc.vector.tensor_scalar(
                out=xt[i],
                in0=xt[i],
                scalar1=sc_all[:, i : i + 1],
                scalar2=sc_all[:, 4 + i : 5 + i],
                op0=mybir.AluOpType.subtract,
                op1=mybir.AluOpType.mult,
            )

        # ---- qkv = W' @ xn + b' ----
        qkv_sb = []
        for j in range(3):
            dst = qkvp.tile([128, N], F16, tag=f"qkv{j}")
            qkv_sb.append(dst)
        for j in range(3):
            for ch in range(8):
                ps = ps_acc.tile([128, 512], F32, tag="acc")
                for kt in range(4):
                    nc.tensor.matmul(
                        ps,
                        lhsT=w_tiles[kt][:, j * 128 : (j + 1) * 128],
                        rhs=xt[kt][:, 512 * ch : 512 * (ch + 1)],
                        start=(kt == 0),
                        stop=(kt == 3),
                    )
                nc.scalar.activation(
                    out=qkv_sb[j][:, 512 * ch : 512 * (ch + 1)],
                    in_=ps,
                    func=mybir.ActivationFunctionType.Identity,
                    bias=bq_sb[:, j : j + 1],
                )
        q_sb, k_sb, v_sb = qkv_sb

        # ---- vT blocks (s on partitions) ----
        vT = qkvp.tile([128, NST, 128], F16, tag="vT")
        for stt in range(NST):
            ps_v = ps_mm2.tile([128, 128], F16, tag="mm2")
            nc.tensor.transpose(
                ps_v, v_sb[:, 128 * stt : 128 * (stt + 1)], ident128_16
            )
            nc.vector.tensor_copy(out=vT[:, stt, :], in_=ps_v)

        # ---- main loop over t chunks ----
        for c in range(NCHUNK):
            t0 = c * TCHUNK
            # scoresT + exp
            ets = []
            for stt in range(NST):
                ps = ps_sc.tile([128, TCHUNK], F32, tag="sc")
                kslice = k_sb[:, 128 * stt : 128 * (stt + 1)]
                for hh in range(2):
                    nc.tensor.matmul(
                        ps[:, 512 * hh : 512 * (hh + 1)],
                        lhsT=kslice,
                        rhs=q_sb[:, t0 + 512 * hh : t0 + 512 * (hh + 1)],
                        start=True,
                        stop=True,
                    )
                et = epool.tile([128, TCHUNK], F16, tag="et")
                nc.scalar.activation(
                    out=et, in_=ps, func=mybir.ActivationFunctionType.Exp
                )
                ets.append(et)

            # Z[t] = sum_s eT[s, t]: fp16 pairwise tree, then ones-matmul
            gsums = []
            for g in range(4):
                e8 = ets[8 * g : 8 * g + 8]
                a = trpool.tile([128, TCHUNK], F16, tag="tr")
                nc.vector.tensor_add(out=a, in0=e8[0], in1=e8[1])
                b_ = trpool.tile([128, TCHUNK], F16, tag="tr")
                nc.vector.tensor_add(out=b_, in0=e8[2], in1=e8[3])
                nc.vector.tensor_add(out=a, in0=a, in1=b_)
                c_ = trpool.tile([128, TCHUNK], F16, tag="tr")
                nc.vector.tensor_add(out=c_, in0=e8[4], in1=e8[5])
                d_ = trpool.tile([128, TCHUNK], F16, tag="tr")
                nc.vector.tensor_add(out=d_, in0=e8[6], in1=e8[7])
                nc.vector.tensor_add(out=c_, in0=c_, in1=d_)
                gsum = gspool.tile([128, TCHUNK], F16, tag="gs")
                nc.vector.tensor_add(out=gsum, in0=a, in1=c_)
                gsums.append(gsum)
            esum = espool.tile([128, TCHUNK], F16, tag="esum")
            nc.vector.tensor_add(out=esum, in0=gsums[0], in1=gsums[1])
            g23 = trpool.tile([128, TCHUNK], F16, tag="tr")
            nc.vector.tensor_add(out=g23, in0=gsums[2], in1=gsums[3])
            nc.vector.tensor_add(out=esum, in0=esum, in1=g23)

            ps_z = ps_sc.tile([1, TCHUNK], F32, tag="sc")
            for hh in range(2):
                nc.tensor.matmul(
                    ps_z[:, 512 * hh : 512 * (hh + 1)],
                    lhsT=ones_col,
                    rhs=esum[:, 512 * hh : 512 * (hh + 1)],
                    start=True,
                    stop=True,
                )
            zinv_row = zpool.tile([1, TCHUNK], F32, tag="zrow")
            nc.vector.reciprocal(out=zinv_row, in_=ps_z)
            zinv_b = zpool.tile([128, TCHUNK], F32, tag="zb")
            nc.gpsimd.partition_broadcast(zinv_b, zinv_row)

            # h = (attn-unnorm @ v) * zinv, then proj + store
            for hh in range(2):
                ps_h = ps_acc.tile([128, 512], F32, tag="acc")
                for stt in range(NST):
                    nc.tensor.matmul(
                        ps_h,
                        lhsT=vT[:, stt, :],
                        rhs=ets[stt][:, 512 * hh : 512 * (hh + 1)],
                        start=(stt == 0),
                        stop=(stt == NST - 1),
                    )
                h_sb = hpool.tile([128, 512], F16, tag="h")
                nc.vector.tensor_mul(
                    out=h_sb, in0=ps_h, in1=zinv_b[:, 512 * hh : 512 * (hh + 1)]
                )
                for ot in range(4):
                    ps_p = ps_mm2.tile([128, 512], F32, tag="mm2")
                    nc.tensor.matmul(
                        ps_p,
                        lhsT=wp_sb[:, 128 * ot : 128 * (ot + 1)],
                        rhs=h_sb,
                        start=True,
                        stop=True,
                    )
                    ob = opool.tile([128, 512], F32, tag="osb")
                    nc.vector.tensor_copy(out=ob, in_=ps_p)
                    nc.sync.dma_start(
                        out=partial[
                            128 * ot : 128 * (ot + 1),
                            t0 + 512 * hh : t0 + 512 * (hh + 1),
                        ],
                        in_=ob,
                    )
    if not nc.is_finalized():
        nc.finalize()
    return nc


_NC_CACHE = None


def _get_nc():
    global _NC_CACHE
    if _NC_CACHE is None:
        _NC_CACHE = build_program()
    return _NC_CACHE


def kernel(x, norm_w, norm_b, w_qkv, w_proj, b_proj):
    global LAST_RESULT
    x = np.asarray(x, dtype=np.float32)
    norm_w = np.asarray(norm_w, dtype=np.float32)
    norm_b = np.asarray(norm_b, dtype=np.float32)
    w_qkv = np.asarray(w_qkv, dtype=np.float32)
    w_proj = np.asarray(w_proj, dtype=np.float32)
    b_proj = np.asarray(b_proj, dtype=np.float32)

    s1 = 1.0 / math.sqrt(math.sqrt(CH))
    in_maps = []
    for core in range(NCORES):
        b, h = divmod(core, NH)
        # reference layout: head h of batch b uses w_qkv rows
        # [384h:384h+128] (q), [384h+128:384h+256] (k), [384h+256:384h+384] (v)
        rows = w_qkv[384 * h : 384 * (h + 1)]  # (384, 512)
        wfold = rows * norm_w[None, :]  # fold GroupNorm gamma
        bias = rows @ norm_b  # fold GroupNorm beta
        scale_vec = np.concatenate(
            [np.full(128, s1), np.full(128, s1), np.ones(128)]
        ).astype(np.float32)
        wfold = wfold * scale_vec[:, None]
        bias = bias * scale_vec
        wqkvT = np.ascontiguousarray(
            wfold.T.reshape(4, 128, 384).astype(np.float16)
        )
        bqkv = np.ascontiguousarray(bias.reshape(3, 128).T.astype(np.float32))
        wprojT = np.ascontiguousarray(
            w_proj[:, 128 * h : 128 * (h + 1)].T.astype(np.float16)
        )
        x16 = np.ascontiguousarray(x[b].reshape(C, N).astype(np.float16))
        mgrp = (np.arange(128)[:, None] // 16 == np.arange(8)[None, :]).astype(
            np.float16
        )
        in_maps.append(
            {
                "x16": x16,
                "wqkvT": wqkvT,
                "bqkv": bqkv,
                "wprojT": wprojT,
                "mgrp": mgrp,
                "mgrpT": np.ascontiguousarray(mgrp.T),
            }
        )

    nc = _get_nc()
    res = run_bass_kernel_spmd(
        nc,
        in_maps,
        list(range(NCORES)),
        trace=TRACE,
        trace_cores=TRACE_CORES if TRACE else None,
    )
    LAST_RESULT = res

    out = np.empty((B, C, N), dtype=np.float32)
    for b in range(B):
        acc = x[b].reshape(C, N) + b_proj[:, None]
        for h in range(NH):
            acc = acc + res.results[4 * b + h]["partial"]
        out[b] = acc
    return out.reshape(B, C, 64, 64)


# revision 23
# speedup vs baseline: 1.2875x; 1.2875x over previous
"""AttentionBlock (GroupNorm -> qkv conv1x1 -> 4-head attention -> proj + residual)
on 8 Trainium2 NeuronCores.

Sharding: B*NH = 2*4 = 8 (batch, head) pairs -> one per core.
Each core:
  - GroupNorm(32, 512) over its batch's x (recomputed per core; vector work)
  - qkv for its head:  q,k,v = W'[3*128, 512] @ xn   (norm affine + qk scale
    folded into W'/bias on host)
  - scoresT[s,t] = sum_c k[c,s] q[c,t]  (s on partitions -> exp output needs
    no transposes).  No max-subtraction: scores are O(1) for this problem.
  - eT = exp(scoresT) in fp16;  Z[t] via fp16 pairwise add-tree + ones-matmul
  - h[c,t] = (sum_s v[c,s] eT[s,t]) * (1/Z[t])
  - partial[o,t] = w_proj[o, head_slice] @ h
Host: out[b] = sum_heads partial + b_proj + x  (gather/unshard).
"""

import math
from contextlib import ExitStack

import ml_dtypes
import numpy as np

import concourse.bacc as bacc
import concourse.bass as bass
import concourse.mybir as mybir
import concourse.tile as tile
from concourse.bass_utils import run_bass_kernel_spmd
from concourse.masks import make_identity

C = 512
NH = 4
G = 32
EPS = 1e-5
N = 4096          # H*W
CH = 128          # channels per head
B = 2
NCORES = 8
TCHUNK = 1024     # t-columns processed per chunk
NCHUNK = N // TCHUNK
NST = N // 128    # number of 128-wide s tiles

F16 = mybir.dt.float16
BF16 = mybir.dt.bfloat16
F32 = mybir.dt.float32

TRACE = False
TRACE_CORES = [0]
LAST_RESULT = None


def build_program():
    nc = bacc.Bacc()

    x16 = nc.declare_dram_parameter("x16", [C, N], BF16, isOutput=False)
    wqkvT = nc.declare_dram_parameter("wqkvT", [4, 128, 3 * CH], BF16, isOutput=False)
    bqkv = nc.declare_dram_parameter("bqkv", [128, 3], F32, isOutput=False)
    wprojT = nc.declare_dram_parameter("wprojT", [CH, C], BF16, isOutput=False)
    # group membership matrices: mgrp[p, g] = (p // 16 == g)
    mgrp = nc.declare_dram_parameter("mgrp", [128, 8], BF16, isOutput=False)
    mgrpT = nc.declare_dram_parameter("mgrpT", [8, 128], BF16, isOutput=False)
    partial = nc.declare_dram_parameter("partial", [C, N], F32, isOutput=True)
    zout = nc.declare_dram_parameter("zout", [1, N], F32, isOutput=True)

    with tile.TileContext(nc) as tc, ExitStack() as ctx:
        consts = ctx.enter_context(tc.tile_pool(name="consts", bufs=1))
        gn = ctx.enter_context(tc.tile_pool(name="gn", bufs=1))
        xpool = ctx.enter_context(tc.tile_pool(name="xpool", bufs=4))
        spool = ctx.enter_context(tc.tile_pool(name="spool", bufs=2))
        qkvp = ctx.enter_context(tc.tile_pool(name="qkvp", bufs=1))
        epool = ctx.enter_context(tc.tile_pool(name="epool", bufs=30))
        trpool = ctx.enter_context(tc.tile_pool(name="trpool", bufs=5))
        gspool = ctx.enter_context(tc.tile_pool(name="gspool", bufs=5))
        espool = ctx.enter_context(tc.tile_pool(name="espool", bufs=2))
        zpool = ctx.enter_context(tc.tile_pool(name="zpool", bufs=2))
        hpool = ctx.enter_context(tc.tile_pool(name="hpool", bufs=3))
        opool = ctx.enter_context(tc.tile_pool(name="opool", bufs=3))
        ps_sc = ctx.enter_context(tc.tile_pool(name="ps_sc", bufs=2, space="PSUM"))
        ps_acc = ctx.enter_context(tc.tile_pool(name="ps_acc", bufs=2, space="PSUM"))
        ps_mm2 = ctx.enter_context(tc.tile_pool(name="ps_mm2", bufs=2, space="PSUM"))

        # ---- constants ----
        ident128_16 = consts.tile([128, 128], BF16, tag="id16")
        make_identity(nc, ident128_16)
        mgrp_sb = consts.tile([128, 8], BF16, tag="mgrp")
        nc.sync.dma_start(out=mgrp_sb, in_=mgrp[:, :])
        mgrpT_sb = consts.tile([8, 128], BF16, tag="mgrpT")
        nc.sync.dma_start(out=mgrpT_sb, in_=mgrpT[:, :])
        ones_col = consts.tile([128, 1], F16, tag="ones")
        nc.vector.memset(ones_col, 1.0)
        eps_sb = consts.tile([128, 1], F32, tag="eps")
        nc.vector.memset(eps_sb, EPS)

        w_tiles = []
        for kt in range(4):
            wt = consts.tile([128, 3 * CH], BF16, tag=f"wq{kt}")
            nc.sync.dma_start(out=wt, in_=wqkvT[kt])
            w_tiles.append(wt)
        bq_sb = consts.tile([128, 3], F32, tag="bq")
        nc.sync.dma_start(out=bq_sb, in_=bqkv[:, :])
        wp_sb = consts.tile([CH, C], BF16, tag="wp")
        nc.sync.dma_start(out=wp_sb, in_=wprojT[:, :])

        # ---- load x tiles + per-channel stats ----
        stats_all = gn.tile([128, 8], F32, tag="stats_all")
        xt = []
        for i in range(4):
            xti = xpool.tile([128, N], BF16, tag="xt")
            nc.sync.dma_start(out=xti, in_=x16[128 * i : 128 * (i + 1), :])
            xt.append(xti)
            st = spool.tile([128, 8, 6], F32, tag="bst")
            xv = xti.rearrange("p (s f) -> p s f", f=512)
            for s in range(8):
                nc.vector.bn_stats(out=st[:, s, :], in_=xv[:, s, :])
            mv = spool.tile([128, 2], F32, tag="mv")
            nc.vector.bn_aggr(out=mv, in_=st)
            # stats_all[:, i] = channel mean;  stats_all[:, 4+i] = E[x^2]
            nc.vector.tensor_copy(out=stats_all[:, i : i + 1], in_=mv[:, 0:1])
            nc.vector.tensor_mul(
                out=stats_all[:, 4 + i : 5 + i], in0=mv[:, 0:1], in1=mv[:, 0:1]
            )
            nc.vector.tensor_add(
                out=stats_all[:, 4 + i : 5 + i],
                in0=stats_all[:, 4 + i : 5 + i],
                in1=mv[:, 1:2],
            )

        # ---- cross-partition group aggregation via PE ----
        # gs[g, col] = sum over partitions p in group g of stats_all[p, col]
        stats16 = gn.tile([128, 8], BF16, tag="stats16")
        nc.vector.tensor_copy(out=stats16, in_=stats_all)
        ps_t = ps_mm2.tile([8, 8], F32, tag="mm2")
        nc.tensor.matmul(ps_t, lhsT=mgrp_sb, rhs=stats16, start=True, stop=True)
        gs = gn.tile([8, 8], F32, tag="gs8")
        nc.scalar.mul(out=gs, in_=ps_t, mul=1.0 / 16.0)
        # gvals cols 0..3 = group mean per x-tile, cols 4..7 = group rstd
        gvals = gn.tile([8, 8], F32, tag="gvals")
        nc.vector.tensor_copy(out=gvals[:, 0:4], in_=gs[:, 0:4])
        varg = gn.tile([8, 4], F32, tag="varg")
        nc.vector.tensor_mul(out=varg, in0=gs[:, 0:4], in1=gs[:, 0:4])  # mu^2
        nc.vector.tensor_sub(out=varg, in0=gs[:, 4:8], in1=varg)  # var
        nc.scalar.activation(
            out=varg,
            in_=varg,
            func=mybir.ActivationFunctionType.Sqrt,
            bias=eps_sb[0:8, :],
        )
        nc.vector.reciprocal(out=gvals[:, 4:8], in_=varg)  # rstd
        # broadcast group values back to all 128 channel partitions
        gvals16 = gn.tile([8, 8], BF16, tag="gvals16")
        nc.vector.tensor_copy(out=gvals16, in_=gvals)
        ps_t2 = ps_mm2.tile([128, 8], F32, tag="mm2")
        nc.tensor.matmul(ps_t2, lhsT=mgrpT_sb, rhs=gvals16, start=True, stop=True)
        sc_all = gn.tile([128, 8], F32, tag="scall")
        nc.vector.tensor_copy(out=sc_all, in_=ps_t2)

        # ---- apply normalization in place: xn = (x - mu) * rstd ----
        for i in range(4):
            nc.vector.tensor_scalar(
                out=xt[i],
                in0=xt[i],
                scalar1=sc_all[:, i : i + 1],
                scalar2=sc_all[:, 4 + i : 5 + i],
                op0=mybir.AluOpType.subtract,
                op1=mybir.AluOpType.mult,
            )

        # ---- qkv = W' @ xn + b' ----
        qkv_sb = []
        for j in range(3):
            dst = qkvp.tile([128, N], BF16, tag=f"qkv{j}")
            qkv_sb.append(dst)
        for j in range(3):
            for ch in range(8):
                ps = ps_acc.tile([128, 512], F32, tag="acc")
                for kt in range(4):
                    nc.tensor.matmul(
                        ps,
                        lhsT=w_tiles[kt][:, j * 128 : (j + 1) * 128],
                        rhs=xt[kt][:, 512 * ch : 512 * (ch + 1)],
                        start=(kt == 0),
                        stop=(kt == 3),
                    )
                nc.scalar.activation(
                    out=qkv_sb[j][:, 512 * ch : 512 * (ch + 1)],
                    in_=ps,
                    func=mybir.ActivationFunctionType.Identity,
                    bias=bq_sb[:, j : j + 1],
                )
        q_sb, k_sb, v_sb = qkv_sb

        # ---- vT blocks (s on partitions) ----
        vT = qkvp.tile([128, NST, 128], BF16, tag="vT")
        for stt in range(NST):
            ps_v = ps_mm2.tile([128, 128], BF16, tag="mm2")
            nc.tensor.transpose(
                ps_v, v_sb[:, 128 * stt : 128 * (stt + 1)], ident128_16
            )
            nc.vector.tensor_copy(out=vT[:, stt, :], in_=ps_v)

        # ---- main loop over t chunks ----
        for c in range(NCHUNK):
            t0 = c * TCHUNK
            # scoresT + exp
            ets = []
            for stt in range(NST):
                ps = ps_sc.tile([128, TCHUNK], F32, tag="sc")
                kslice = k_sb[:, 128 * stt : 128 * (stt + 1)]
                for hh in range(2):
                    nc.tensor.matmul(
                        ps[:, 512 * hh : 512 * (hh + 1)],
                        lhsT=kslice,
                        rhs=q_sb[:, t0 + 512 * hh : t0 + 512 * (hh + 1)],
                        start=True,
                        stop=True,
                    )
                et = epool.tile([128, TCHUNK], BF16, tag="et")
                nc.scalar.activation(
                    out=et, in_=ps, func=mybir.ActivationFunctionType.Exp
                )
                ets.append(et)

            # Z[t] = sum_s eT[s, t]: fp16 pairwise tree, then ones-matmul
            gsums = []
            for g in range(4):
                e8 = ets[8 * g : 8 * g + 8]
                a = trpool.tile([128, TCHUNK], F16, tag="tr")
                nc.vector.tensor_add(out=a, in0=e8[0], in1=e8[1])
                b_ = trpool.tile([128, TCHUNK], F16, tag="tr")
                nc.vector.tensor_add(out=b_, in0=e8[2], in1=e8[3])
                nc.vector.tensor_add(out=a, in0=a, in1=b_)
                c_ = trpool.tile([128, TCHUNK], F16, tag="tr")
                nc.vector.tensor_add(out=c_, in0=e8[4], in1=e8[5])
                d_ = trpool.tile([128, TCHUNK], F16, tag="tr")
                nc.vector.tensor_add(out=d_, in0=e8[6], in1=e8[7])
                nc.vector.tensor_add(out=c_, in0=c_, in1=d_)
                gsum = gspool.tile([128, TCHUNK], F16, tag="gs")
                nc.vector.tensor_add(out=gsum, in0=a, in1=c_)
                gsums.append(gsum)
            esum = espool.tile([128, TCHUNK], F16, tag="esum")
            nc.vector.tensor_add(out=esum, in0=gsums[0], in1=gsums[1])
            g23 = trpool.tile([128, TCHUNK], F16, tag="tr")
            nc.vector.tensor_add(out=g23, in0=gsums[2], in1=gsums[3])
            nc.vector.tensor_add(out=esum, in0=esum, in1=g23)

            ps_z = ps_sc.tile([1, TCHUNK], F32, tag="sc")
            for hh in range(2):
                nc.tensor.matmul(
                    ps_z[:, 512 * hh : 512 * (hh + 1)],
                    lhsT=ones_col,
                    rhs=esum[:, 512 * hh : 512 * (hh + 1)],
                    start=True,
                    stop=True,
                )
            zrow = zpool.tile([1, TCHUNK], F32, tag="zrow")
            nc.vector.tensor_copy(out=zrow, in_=ps_z)
            nc.sync.dma_start(out=zout[:, t0 : t0 + TCHUNK], in_=zrow)

            # h_unnorm = attn-unnorm @ v, then proj + store (host divides by Z)
            for hh in range(2):
                ps_h = ps_acc.tile([128, 512], F32, tag="acc")
                for stt in range(NST):
                    nc.tensor.matmul(
                        ps_h,
                        lhsT=vT[:, stt, :],
                        rhs=ets[stt][:, 512 * hh : 512 * (hh + 1)],
                        start=(stt == 0),
                        stop=(stt == NST - 1),
                    )
                h_sb = hpool.tile([128, 512], BF16, tag="h")
                nc.vector.tensor_copy(out=h_sb, in_=ps_h)
                for ot in range(4):
                    ps_p = ps_mm2.tile([128, 512], F32, tag="mm2")
                    nc.tensor.matmul(
                        ps_p,
                        lhsT=wp_sb[:, 128 * ot : 128 * (ot + 1)],
                        rhs=h_sb,
                        start=True,
                        stop=True,
                    )
                    ob = opool.tile([128, 512], F32, tag="osb")
                    nc.vector.tensor_copy(out=ob, in_=ps_p)
                    nc.sync.dma_start(
                        out=partial[
                            128 * ot : 128 * (ot + 1),
                            t0 + 512 * hh : t0 + 512 * (hh + 1),
                        ],
                        in_=ob,
                    )
    if not nc.is_finalized():
        nc.finalize()
    return nc


_NC_CACHE = None


def _get_nc():
    global _NC_CACHE
    if _NC_CACHE is None:
        _NC_CACHE = build_program()
    return _NC_CACHE


def kernel(x, norm_w, norm_b, w_qkv, w_proj, b_proj):
    global LAST_RESULT
    x = np.asarray(x, dtype=np.float32)
    norm_w = np.asarray(norm_w, dtype=np.float32)
    norm_b = np.asarray(norm_b, dtype=np.float32)
    w_qkv = np.asarray(w_qkv, dtype=np.float32)
    w_proj = np.asarray(w_proj, dtype=np.float32)
    b_proj = np.asarray(b_proj, dtype=np.float32)

    s1 = 1.0 / math.sqrt(math.sqrt(CH))
    in_maps = []
    for core in range(NCORES):
        b, h = divmod(core, NH)
        # reference layout: head h of batch b uses w_qkv rows
        # [384h:384h+128] (q), [384h+128:384h+256] (k), [384h+256:384h+384] (v)
        rows = w_qkv[384 * h : 384 * (h + 1)]  # (384, 512)
        wfold = rows * norm_w[None, :]  # fold GroupNorm gamma
        bias = rows @ norm_b  # fold GroupNorm beta
        scale_vec = np.concatenate(
            [np.full(128, s1), np.full(128, s1), np.ones(128)]
        ).astype(np.float32)
        wfold = wfold * scale_vec[:, None]
        bias = bias * scale_vec
        bf16 = ml_dtypes.bfloat16
        wqkvT = np.ascontiguousarray(
            wfold.T.reshape(4, 128, 384).astype(bf16)
        )
        bqkv = np.ascontiguousarray(bias.reshape(3, 128).T.astype(np.float32))
        wprojT = np.ascontiguousarray(
            w_proj[:, 128 * h : 128 * (h + 1)].T.astype(bf16)
        )
        x16 = np.ascontiguousarray(x[b].reshape(C, N).astype(bf16))
        mgrp = (np.arange(128)[:, None] // 16 == np.arange(8)[None, :]).astype(
            bf16
        )
        in_maps.append(
            {
                "x16": x16,
                "wqkvT": wqkvT,
                "bqkv": bqkv,
                "wprojT": wprojT,
                "mgrp": mgrp,
                "mgrpT": np.ascontiguousarray(mgrp.T),
            }
        )

    nc = _get_nc()
    res = run_bass_kernel_spmd(
        nc,
        in_maps,
        list(range(NCORES)),
        trace=TRACE,
        trace_cores=TRACE_CORES if TRACE else None,
    )
    LAST_RESULT = res

    out = np.empty((B, C, N), dtype=np.float32)
    for b in range(B):
        acc = x[b].reshape(C, N) + b_proj[:, None]
        for h in range(NH):
            r = res.results[4 * b + h]
            acc = acc + r["partial"] / r["zout"]
        out[b] = acc
    return out.reshape(B, C, 64, 64)
